# revision 20
# baseline (speedup 1.0000x reference)
"""Cross-attention kernel for Trainium2, 8 NeuronCores, data-parallel over batch.

Reference computation (per batch b):
  lq = Wl @ lb + bl          [D, N]   (D == N == 256)
  fk = Wf @ fm + bf          [D, N]
  v  = Wv @ fm + bv          [C, N]
  att = softmax(lq @ fk.T)   [D, D]   (softmax over last dim)
  out = v @ att.T + fm       [C, N]

Strategy: batch 64 is split 8 ways (8 batches per core). All matmuls are
emitted with the activation tile as the stationary operand (lhsT) and the
weights streaming, computing transposed projections directly:
  fkT[n, d] = sum_c fm[c, n] WfT[c, d]     (fp32r / FP22)
  lqT[n, d] = sum_l lb[l, n] WlT[l, d]     (fp32r)
  logits[d, e] = sum_n lqT[n, d] fkT[n, e] (fp32r)
  vT[n, c] = sum_c' fm[c', n] WvT[c', c]   (bf16 k-tiles 0..7, fp8e4 DoubleRow
                                            k-tiles 8..15: 2 k-tiles/matmul)
  out[c, d] = sum_n vT[n, c] attT[n, d]    (bf16)
The value contraction is split by precision: channels 0..K16*128-1 in bf16,
the rest in fp8e4 DoubleRow (~2x PE rate). Both weight slices are
pre-scaled by 32 on the host (keeps fp8 weights out of the subnormal range);
the PSUM->SBUF copy of vT divides by 32. Rel-err budget: fp8 on 12/16 of
the contraction measures ~1.85e-2 (vs 2e-2 gate). Biases bl/bf enter via rank-1
augmentation matmuls (ones x bias row); bv is added in the epilogue (exact:
softmax rows sum to 1). Residual add is fused into the PSUM->SBUF epilogue.
"""

import numpy as np
import ml_dtypes

import concourse.bass as bass
import concourse.mybir as mybir
import concourse.tile as tile
from concourse import bacc
from concourse.bass_utils import run_bass_kernel_spmd
from concourse.masks import make_identity

N_CORES = 8
B = 64
C = 2048
L = 512
HW = 256          # N = H*W, == D
D = HW
P = 128
B_SHARD = B // N_CORES

F32 = mybir.dt.float32
F32R = mybir.dt.float32r
BF16 = mybir.dt.bfloat16
FP16 = mybir.dt.float16
FP8 = mybir.dt.float8e4

CK = C // P       # 16 k-tiles over channel contraction
K16 = 4           # k-tiles computed in bf16 (channels 0..511)
K8 = CK - K16     # k-tiles computed in fp8e4 DoubleRow (channels 512..2047)
LK = L // P       # 4 k-tiles over label contraction
NT = HW // P      # 2 tiles over spatial/projection dim
CM = C // P       # 16 output-channel chunks
VC = 512          # value-matmul free-dim chunk (one PSUM bank of fp32)
VSCALE = 32.0     # host-side pre-scale on Wv halves (undone at vT copy)
DR = mybir.MatmulPerfMode.DoubleRow


def build_kernel(b_shard=B_SHARD, with_bias=True, warm_mms=100):
    nc = bacc.Bacc("TRN2", target_bir_lowering=False, debug=False,
                   num_devices=N_CORES)

    # all tensors host-pre-transposed to [partition, ktile, free] so each DMA
    # is 128 large contiguous lines (descriptor-generation cost on the issuing
    # engine is ~2.4ns/line; fragmented layouts stall the DGE rings)
    fm_d = nc.dram_tensor("fm", [b_shard, P, CK, HW], FP16, kind="ExternalInput")
    lb_d = nc.dram_tensor("lb", [b_shard, P, LK, HW], FP16, kind="ExternalInput")
    wft_d = nc.dram_tensor("wft", [P, CK, D], FP16, kind="ExternalInput")
    wlt_d = nc.dram_tensor("wlt", [P, LK, D], FP16, kind="ExternalInput")
    wvt16_d = nc.dram_tensor("wvt16", [P, K16, C], BF16, kind="ExternalInput")
    wvt8_d = nc.dram_tensor("wvt8", [P, K8, C], FP8, kind="ExternalInput")
    bf_d = nc.dram_tensor("bfc", [1, D], F32R, kind="ExternalInput")
    bl_d = nc.dram_tensor("blc", [1, D], F32R, kind="ExternalInput")
    bv_d = nc.dram_tensor("bvc", [P, CM], F32, kind="ExternalInput")
    ones_d = nc.dram_tensor("ones", [1, P], F32R, kind="ExternalInput")
    out_d = nc.dram_tensor("out", [b_shard, P, CK, HW], F32, kind="ExternalOutput")

    with tile.TileContext(nc) as tc:
        with (
            tc.tile_pool(name="wpool", bufs=1) as wpool,
            tc.tile_pool(name="fmp", bufs=2) as fmp,
            tc.tile_pool(name="fmb", bufs=2) as fmb,
            tc.tile_pool(name="lbp", bufs=2) as lbp,
            tc.tile_pool(name="proj", bufs=2) as proj,
            tc.tile_pool(name="attp", bufs=2) as attp,
            tc.tile_pool(name="valp", bufs=2) as valp,
            tc.tile_pool(name="outp", bufs=8) as outp,
            tc.tile_pool(name="stat", bufs=2) as stat,
            tc.tile_pool(name="ps_small", bufs=3, space="PSUM") as ps_small,
            tc.tile_pool(name="ps_val", bufs=4, space="PSUM") as ps_val,
            tc.tile_pool(name="ps_att", bufs=1, space="PSUM") as ps_att,
        ):
            # ---- resident weights / constants ----
            wft = wpool.tile([P, CK, D], FP16)
            wlt = wpool.tile([P, LK, D], FP16)
            wvt16 = wpool.tile([P, K16, C], BF16)
            wvt8 = wpool.tile([P, K8, C], FP8)
            bfb = wpool.tile([1, D], F32R)
            blb = wpool.tile([1, D], F32R)
            bvb = wpool.tile([P, CM], F32)
            ones = wpool.tile([1, P], F32R)
            ident = wpool.tile([P, P], BF16)

            if with_bias:
                nc.sync.dma_start(ones[:], ones_d.ap())
                nc.sync.dma_start(bfb[:], bf_d.ap())
                nc.sync.dma_start(blb[:], bl_d.ap())
            make_identity(nc, ident[:])

            fms = {}    # b -> (fm f32r, fm16 bf16 [0:K16], fm8 fp8 [K16:CK])
            atts = {}   # b -> attT tile

            def load(b, split=False, eng=None):
                eng = eng or nc.sync
                fm = fmp.tile([P, CK, HW], FP16)
                lbt = lbp.tile([P, LK, HW], FP16)
                fm16 = fmb.tile([P, K16, HW], BF16, tag="fm16", name="fm16")
                fm8 = fmb.tile([P, K8, HW], FP8, tag="fm8", name="fm8")
                if split:
                    # batch 0: land the bf16 slice of fm first so the value
                    # GEMM starts while the rest of the startup DMA drains;
                    # the fp8 slice lands in 2 halves so DoubleRow matmuls
                    # chase the DMA instead of waiting for the whole tensor
                    eng.dma_start(fm[:, 0:K16, :], fm_d[b][:, 0:K16, :])
                    eng.dma_start(lbt[:], lb_d[b])
                    eng.dma_start(wlt[:], wlt_d.ap())
                    nc.scalar.copy(fm16[:], fm[:, 0:K16, :])
                    half = K8 // 2
                    for h in range(2):
                        sl = slice(K16 + h * half, K16 + (h + 1) * half)
                        eng.dma_start(fm[:, sl, :], fm_d[b][:, sl, :])
                        nc.scalar.copy(fm8[:, h * half:(h + 1) * half, :],
                                       fm[:, sl, :])
                else:
                    eng.dma_start(lbt[:], lb_d[b])
                    eng.dma_start(fm[:], fm_d[b])
                    # conversions on ScalarE: keeps the DVE free for the
                    # epilogue chain that paces the inter-batch handoff
                    nc.scalar.copy(fm16[:], fm[:, 0:K16, :])
                    nc.scalar.copy(fm8[:], fm[:, K16:CK, :])
                fms[b] = (fm, fm16, fm8)
                return lbt

            def att_lqt(b, lbt):
                lqt = proj.tile([P, NT, D], F32R, tag="lqt", name="lqt")
                for nt in range(NT):
                    ps = ps_small.tile([P, D], F32, tag="ps", name="ps")
                    for k in range(LK):
                        nc.tensor.matmul(
                            ps[:], lbt[:, k, nt * P:(nt + 1) * P], wlt[:, k, :],
                            start=(k == 0),
                            stop=(not with_bias and k == LK - 1))
                    if with_bias:
                        nc.tensor.matmul(ps[:], ones[:], blb[:], start=False,
                                         stop=True)
                    nc.vector.tensor_copy(lqt[:, nt, :], ps[:])
                return lqt

            def att_fkt(b, nt, fkt=None):
                if fkt is None:
                    fkt = proj.tile([P, NT, D], F32R, tag="fkt", name="fkt")
                fm = fms[b][0]
                ps = ps_small.tile([P, D], F32, tag="ps", name="ps")
                for k in range(CK):
                    nc.tensor.matmul(
                        ps[:], fm[:, k, nt * P:(nt + 1) * P], wft[:, k, :],
                        start=(k == 0),
                        stop=(not with_bias and k == CK - 1))
                if with_bias:
                    nc.tensor.matmul(ps[:], ones[:], bfb[:], start=False,
                                     stop=True)
                nc.vector.tensor_copy(fkt[:, nt, :], ps[:])
                return fkt

            def att_softmax(b, lqt, fkt):
                att = attp.tile([P, NT, D], BF16, tag="att", name="att")
                negmax = stat.tile([P, NT], F32, tag="negmax", name="negmax")
                sumexp = stat.tile([P, NT], F32, tag="sumexp", name="sumexp")
                recip = stat.tile([P, NT], F32, tag="recip", name="recip")
                for dm in range(NT):
                    ps = ps_small.tile([P, D], F32, tag="ps", name="ps")
                    for kn in range(NT):
                        nc.tensor.matmul(
                            ps[:], lqt[:, kn, dm * P:(dm + 1) * P], fkt[:, kn, :],
                            start=(kn == 0), stop=(kn == NT - 1))
                    nc.vector.tensor_reduce(
                        negmax[:, dm:dm + 1], ps[:], axis=mybir.AxisListType.X,
                        op=mybir.AluOpType.max, negate=True)
                    nc.scalar.activation(
                        att[:, dm, :], ps[:], mybir.ActivationFunctionType.Exp,
                        bias=negmax[:, dm:dm + 1], scale=1.0,
                        accum_out=sumexp[:, dm:dm + 1])
                    nc.vector.reciprocal(recip[:, dm:dm + 1], sumexp[:, dm:dm + 1])
                    nc.vector.tensor_scalar_mul(
                        att[:, dm, :], att[:, dm, :], recip[:, dm:dm + 1])

                attT = attp.tile([P, NT, D], BF16, tag="attT", name="attT")
                for et in range(NT):
                    psT = ps_att.tile([P, D], BF16, name="psT")
                    for dt_ in range(NT):
                        nc.tensor.transpose(
                            psT[:, dt_ * P:(dt_ + 1) * P],
                            att[:, dt_, et * P:(et + 1) * P], ident[:])
                    nc.scalar.copy(attT[:, et, :], psT[:])
                atts[b] = attT

            def att_path(b, lbt):
                """fkT/lqT/logits/softmax/transpose -> attT (PE: ~5us)."""
                lqt = att_lqt(b, lbt)
                fkt = att_fkt(b, 0)
                att_fkt(b, 1, fkt)
                att_softmax(b, lqt, fkt)

            def final_pair(b, fm, vt, attT, cm, dma_eng=None):
                """out chunks cm, cm+1: matmuls + fused epilogue + one DMA."""
                dma_eng = dma_eng or nc.gpsimd
                ost = outp.tile([P, 2, D], F32, name="ost")
                for j in range(2):
                    ps = ps_small.tile([P, D], F32, tag="ps", name="fps")
                    for kn in range(NT):
                        nc.tensor.matmul(
                            ps[:], vt[:, kn, (cm + j) * P:(cm + j + 1) * P],
                            attT[:, kn, :],
                            start=(kn == 0), stop=(kn == NT - 1))
                    nc.vector.scalar_tensor_tensor(
                        ost[:, j, :], ps[:], bvb[:, cm + j:cm + j + 1],
                        fm[:, cm + j, :],
                        op0=mybir.AluOpType.add, op1=mybir.AluOpType.add)
                dma_eng.dma_start(out_d[b][:, cm:cm + 2, :], ost[:])

            def vt_copy(vt, nt, cc, ps):
                """PSUM -> SBUF bf16 with the 1/VSCALE compensation; alternate
                ScalarE / DVE so consecutive bank drains run in parallel."""
                dst = vt[:, nt, cc * VC:(cc + 1) * VC]
                if cc % 2 == 0:
                    nc.scalar.mul(dst, ps[:], 1.0 / VSCALE)
                else:
                    nc.vector.tensor_scalar_mul(dst, ps[:], 1.0 / VSCALE)

            def value_gemm_nt(fm16, fm8, vt, nt):
                """vT[:, nt, :]: k-outer / cc-inner so each stationary
                activation tile is reused across the 4 cc chunks (amortizes
                LDWEIGHTS, critical for the fp8 DoubleRow section whose
                256-col weight loads are 2x the bf16 cost)."""
                pss = [ps_val.tile([P, VC], F32, tag="vps", name=f"vps{i}")
                       for i in range(C // VC)]
                for k in range(K16):
                    for cc in range(C // VC):
                        nc.tensor.matmul(
                            pss[cc][:], fm16[:, k, nt * P:(nt + 1) * P],
                            wvt16[:, k, cc * VC:(cc + 1) * VC],
                            start=(k == 0), stop=False)
                for kp in range(K8 // 2):
                    for cc in range(C // VC):
                        nc.tensor.matmul(
                            pss[cc][:],
                            fm8[:, 2 * kp:2 * kp + 2, nt * P:(nt + 1) * P],
                            wvt8[:, 2 * kp:2 * kp + 2, cc * VC:(cc + 1) * VC],
                            start=False, stop=(kp == K8 // 2 - 1),
                            perf_mode=DR)
                for cc in range(C // VC):
                    vt_copy(vt, nt, cc, pss[cc])

            def value_out(b, fm, vt, attT):
                """out = vT.T @ attT + bv + residual, interleaved with the
                next batch's attention path when available."""
                nxt = interleave.pop(b, None)
                if nxt is None:
                    for cm in range(0, CM, 2):
                        final_pair(b, fm, vt, attT, cm)
                else:
                    # interleave final pairs (DVE-chain-paced) with the next
                    # batch's attention matmuls (PE-paced) to fill PE stalls
                    nb, nlbt = nxt
                    final_pair(b, fm, vt, attT, 0)
                    final_pair(b, fm, vt, attT, 2)
                    lqt = att_lqt(nb, nlbt)
                    final_pair(b, fm, vt, attT, 4)
                    final_pair(b, fm, vt, attT, 6)
                    fkt = att_fkt(nb, 0)
                    final_pair(b, fm, vt, attT, 8)
                    final_pair(b, fm, vt, attT, 10)
                    att_fkt(nb, 1, fkt)
                    final_pair(b, fm, vt, attT, 12)
                    final_pair(b, fm, vt, attT, 14)
                    att_softmax(nb, lqt, fkt)

            def value_final(b):
                """vT (big GEMM) + out epilogue."""
                fm, fm16, fm8 = fms.pop(b)
                attT = atts.pop(b)
                vt = valp.tile([P, NT, C], BF16, name="vt")
                for nt in range(NT):
                    value_gemm_nt(fm16, fm8, vt, nt)
                value_out(b, fm, vt, attT)

            def value_final_tail(b):
                """Last batch: cc-outer so each vT column chunk finishes early
                and its out chunks interleave with the remaining value GEMM,
                shrinking the serial tail."""
                fm, fm16, fm8 = fms.pop(b)
                attT = atts.pop(b)
                vt = valp.tile([P, NT, C], BF16, name="vt")
                chunks = [(cc * VC, VC) for cc in range(C // VC)]
                engs = [nc.sync, nc.scalar, nc.gpsimd]
                ei = 0
                for off, width in chunks:
                    pss = [ps_val.tile([P, width], F32, tag="vps",
                                       name=f"vps{i}") for i in range(NT)]
                    for k in range(K16):
                        for nt in range(NT):
                            nc.tensor.matmul(
                                pss[nt][:], fm16[:, k, nt * P:(nt + 1) * P],
                                wvt16[:, k, off:off + width],
                                start=(k == 0), stop=False)
                    for kp in range(K8 // 2):
                        for nt in range(NT):
                            nc.tensor.matmul(
                                pss[nt][:],
                                fm8[:, 2 * kp:2 * kp + 2, nt * P:(nt + 1) * P],
                                wvt8[:, 2 * kp:2 * kp + 2, off:off + width],
                                start=False, stop=(kp == K8 // 2 - 1),
                                perf_mode=DR)
                    for nt in range(NT):
                        dst = vt[:, nt, off:off + width]
                        if nt % 2 == 0:
                            nc.scalar.mul(dst, pss[nt][:], 1.0 / VSCALE)
                        else:
                            nc.vector.tensor_scalar_mul(dst, pss[nt][:],
                                                        1.0 / VSCALE)
                    for cm in range(off // P, (off + width) // P, 2):
                        # spread descriptor generation over idle engines so
                        # the drain chain is not serialized on one sequencer
                        final_pair(b, fm, vt, attT, cm, dma_eng=engs[ei % 3])
                        ei += 1

            # software pipeline: attention path runs one batch ahead of the
            # big value GEMM so PE never waits on softmax. Batch 0's value
            # GEMM is emitted first and chases the weight DMA chunk-by-chunk;
            # batch 1's inputs ride the gpsimd ring BEHIND the weights so
            # they do not steal bandwidth from the startup critical path.
            lbt0 = load(0, split=True)
            nc.sync.dma_start(bvb[:], bv_d.ap())
            for k in range(K16):
                nc.gpsimd.dma_start(wvt16[:, k, :], wvt16_d.ap()[:, k, :])
            for kp in range(K8 // 2):
                nc.gpsimd.dma_start(wvt8[:, 2 * kp:2 * kp + 2, :],
                                    wvt8_d.ap()[:, 2 * kp:2 * kp + 2, :])
            nc.sync.dma_start(wft[:], wft_d.ap())
            interleave = {}
            lbt1 = load(1) if b_shard > 1 else None
            # warm the PE HAM clock gate during the initial weight-DMA window.
            # K=128 matmuls on the identity tile keep the full array active
            # (K=1 dummies do not register as PE-busy for the HAM).
            warm = ps_att.tile([P, P], F32, tag="psT", name="warm")
            for _ in range(warm_mms if b_shard > 1 else 0):
                nc.tensor.matmul(warm[:], ident[:], ident[:, :P], start=True,
                                 stop=True)
            # batch 0 unrolled: value GEMM before the attention path so the
            # PE consumes weight chunks the moment they land
            fm0, fm016, fm08 = fms[0]
            vt0 = valp.tile([P, NT, C], BF16, name="vt")
            for nt in range(NT):
                value_gemm_nt(fm016, fm08, vt0, nt)
            att_path(0, lbt0)
            if b_shard == 1:
                fm, _, _ = fms.pop(0)
                value_out(0, fm, vt0, atts.pop(0))
            else:
                interleave[0] = (1, lbt1)
                fm, _, _ = fms.pop(0)
                value_out(0, fm, vt0, atts.pop(0))
                for b in range(1, b_shard):
                    if b + 1 < b_shard:
                        lbt = load(b + 1)
                        interleave[b] = (b + 1, lbt)
                    if b == b_shard - 1:
                        value_final_tail(b)
                    else:
                        value_final(b)

    nc.compile()
    return nc


_NC_CACHE = {}


def _get_nc(b_shard, with_bias=True):
    key = (b_shard, with_bias)
    if key not in _NC_CACHE:
        _NC_CACHE[key] = build_kernel(b_shard, with_bias=with_bias)
    return _NC_CACHE[key]


def make_in_maps(feature_maps, labels, Wf, bf, Wl, bl, Wv, bv, b_shard=B_SHARD,
                 n_cores=N_CORES):
    def to_pkf(a, kt):
        # [rows=kt*P, free] -> [P, kt, free], partition-major for 1-line DMAs
        return np.ascontiguousarray(
            a.reshape(kt, P, a.shape[-1]).transpose(1, 0, 2))

    fm = np.asarray(feature_maps, dtype=np.float32).reshape(B, C, HW)
    fm = np.ascontiguousarray(
        fm.reshape(B, CK, P, HW).transpose(0, 2, 1, 3)).astype(np.float16)
    lb = np.asarray(labels, dtype=np.float32).reshape(B, L, HW)
    lb = np.ascontiguousarray(
        lb.reshape(B, LK, P, HW).transpose(0, 2, 1, 3)).astype(np.float16)
    wft = to_pkf(np.asarray(Wf, dtype=np.float32).T.astype(np.float16), CK)
    wlt = to_pkf(np.asarray(Wl, dtype=np.float32).T.astype(np.float16), LK)
    wvs = np.asarray(Wv, dtype=np.float32).T * VSCALE   # [c, o], pre-scaled
    wvt16 = to_pkf(wvs[:K16 * P].astype(ml_dtypes.bfloat16), K16)
    wvt8 = to_pkf(wvs[K16 * P:].astype(ml_dtypes.float8_e4m3), K8)
    bfr = np.asarray(bf, dtype=np.float32).reshape(1, D)
    blr = np.asarray(bl, dtype=np.float32).reshape(1, D)
    bvr = np.ascontiguousarray(
        np.asarray(bv, dtype=np.float32).reshape(CM, P).T)
    in_maps = []
    for i in range(n_cores):
        s = slice(i * b_shard, (i + 1) * b_shard)
        in_maps.append({
            "fm": fm[s], "lb": lb[s], "wft": wft, "wlt": wlt,
            "wvt16": wvt16, "wvt8": wvt8,
            "bfc": bfr, "blc": blr, "bvc": bvr,
            "ones": np.ones((1, P), dtype=np.float32),
        })
    return in_maps


def kernel(feature_maps, labels, Wf, bf, Wl, bl, Wv, bv, _trace=False,
           _tmpdir=None):
    with_bias = bool(np.any(np.asarray(bf)) or np.any(np.asarray(bl)))
    nc = _get_nc(B_SHARD, with_bias)
    in_maps = make_in_maps(feature_maps, labels, Wf, bf, Wl, bl, Wv, bv)
    res = run_bass_kernel_spmd(nc, in_maps, core_ids=list(range(N_CORES)),
                               trace=_trace, tmpdir=_tmpdir)
    out = np.concatenate([res.results[i]["out"] for i in range(N_CORES)], axis=0)
    kernel.last_exec_time_ns = res.exec_time_ns
    # [B, P, CK, HW] -> [B, C, H, W]
    out = out.transpose(0, 2, 1, 3).reshape(B, C, 16, 16)
    return np.ascontiguousarray(out).astype(np.float32)


# revision 21
# speedup vs baseline: 1.0098x; 1.0098x over previous
"""Cross-attention kernel for Trainium2, 8 NeuronCores, data-parallel over batch.

Reference computation (per batch b):
  lq = Wl @ lb + bl          [D, N]   (D == N == 256)
  fk = Wf @ fm + bf          [D, N]
  v  = Wv @ fm + bv          [C, N]
  att = softmax(lq @ fk.T)   [D, D]   (softmax over last dim)
  out = v @ att.T + fm       [C, N]

Strategy: batch 64 is split 8 ways (8 batches per core). All matmuls are
emitted with the activation tile as the stationary operand (lhsT) and the
weights streaming, computing transposed projections directly:
  fkT[n, d] = sum_c fm[c, n] WfT[c, d]     (fp32r / FP22)
  lqT[n, d] = sum_l lb[l, n] WlT[l, d]     (fp32r)
  logits[d, e] = sum_n lqT[n, d] fkT[n, e] (fp32r)
  vT[n, c] = sum_c' fm[c', n] WvT[c', c]   (bf16 k-tiles 0..7, fp8e4 DoubleRow
                                            k-tiles 8..15: 2 k-tiles/matmul)
  out[c, d] = sum_n vT[n, c] attT[n, d]    (bf16)
The value contraction is split by precision: channels 0..K16*128-1 in bf16,
the rest in fp8e4 DoubleRow (~2x PE rate). Both weight slices are
pre-scaled by 32 on the host (keeps fp8 weights out of the subnormal range);
the PSUM->SBUF copy of vT divides by 32. Rel-err budget: fp8 on 12/16 of
the contraction measures ~1.85e-2 (vs 2e-2 gate). Biases bl/bf enter via rank-1
augmentation matmuls (ones x bias row); bv is added in the epilogue (exact:
softmax rows sum to 1). Residual add is fused into the PSUM->SBUF epilogue.
"""

import numpy as np
import ml_dtypes

import concourse.bass as bass
import concourse.mybir as mybir
import concourse.tile as tile
from concourse import bacc
from concourse.bass_utils import run_bass_kernel_spmd
from concourse.masks import make_identity

N_CORES = 8
B = 64
C = 2048
L = 512
HW = 256          # N = H*W, == D
D = HW
P = 128
B_SHARD = B // N_CORES

F32 = mybir.dt.float32
F32R = mybir.dt.float32r
BF16 = mybir.dt.bfloat16
FP16 = mybir.dt.float16
FP8 = mybir.dt.float8e4

CK = C // P       # 16 k-tiles over channel contraction
K16 = 4           # k-tiles computed in bf16 (channels 0..511)
K8 = CK - K16     # k-tiles computed in fp8e4 DoubleRow (channels 512..2047)
LK = L // P       # 4 k-tiles over label contraction
NT = HW // P      # 2 tiles over spatial/projection dim
CM = C // P       # 16 output-channel chunks
VC = 512          # value-matmul free-dim chunk (one PSUM bank of fp32)
VSCALE = 32.0     # host-side pre-scale on Wv halves (undone at vT copy)
DR = mybir.MatmulPerfMode.DoubleRow


def build_kernel(b_shard=B_SHARD, with_bias=True, warm_mms=100):
    nc = bacc.Bacc("TRN2", target_bir_lowering=False, debug=False,
                   num_devices=N_CORES)

    # all tensors host-pre-transposed to [partition, ktile, free] so each DMA
    # is 128 large contiguous lines (descriptor-generation cost on the issuing
    # engine is ~2.4ns/line; fragmented layouts stall the DGE rings)
    fm_d = nc.dram_tensor("fm", [b_shard, P, CK, HW], FP16, kind="ExternalInput")
    lb_d = nc.dram_tensor("lb", [b_shard, P, LK, HW], FP16, kind="ExternalInput")
    wft_d = nc.dram_tensor("wft", [P, CK, D], FP16, kind="ExternalInput")
    wlt_d = nc.dram_tensor("wlt", [P, LK, D], FP16, kind="ExternalInput")
    wvt16_d = nc.dram_tensor("wvt16", [P, K16, C], BF16, kind="ExternalInput")
    wvt8_d = nc.dram_tensor("wvt8", [P, K8, C], FP8, kind="ExternalInput")
    bf_d = nc.dram_tensor("bfc", [1, D], F32R, kind="ExternalInput")
    bl_d = nc.dram_tensor("blc", [1, D], F32R, kind="ExternalInput")
    bv_d = nc.dram_tensor("bvc", [P, CM], F32, kind="ExternalInput")
    ones_d = nc.dram_tensor("ones", [1, P], F32R, kind="ExternalInput")
    out_d = nc.dram_tensor("out", [b_shard, P, CK, HW], F32, kind="ExternalOutput")

    with tile.TileContext(nc) as tc:
        with (
            tc.tile_pool(name="wpool", bufs=1) as wpool,
            tc.tile_pool(name="fmp", bufs=2) as fmp,
            tc.tile_pool(name="fmb", bufs=2) as fmb,
            tc.tile_pool(name="lbp", bufs=2) as lbp,
            tc.tile_pool(name="proj", bufs=2) as proj,
            tc.tile_pool(name="attp", bufs=2) as attp,
            tc.tile_pool(name="valp", bufs=2) as valp,
            tc.tile_pool(name="outp", bufs=8) as outp,
            tc.tile_pool(name="stat", bufs=2) as stat,
            tc.tile_pool(name="ps_small", bufs=3, space="PSUM") as ps_small,
            tc.tile_pool(name="ps_val", bufs=4, space="PSUM") as ps_val,
            tc.tile_pool(name="ps_att", bufs=1, space="PSUM") as ps_att,
        ):
            # ---- resident weights / constants ----
            wft = wpool.tile([P, CK, D], FP16)
            wlt = wpool.tile([P, LK, D], FP16)
            wvt16 = wpool.tile([P, K16, C], BF16)
            wvt8 = wpool.tile([P, K8, C], FP8)
            bfb = wpool.tile([1, D], F32R)
            blb = wpool.tile([1, D], F32R)
            bvb = wpool.tile([P, CM], F32)
            ones = wpool.tile([1, P], F32R)
            ident = wpool.tile([P, P], BF16)

            if with_bias:
                nc.sync.dma_start(ones[:], ones_d.ap())
                nc.sync.dma_start(bfb[:], bf_d.ap())
                nc.sync.dma_start(blb[:], bl_d.ap())
            make_identity(nc, ident[:])

            fms = {}    # b -> (fm f32r, fm16 bf16 [0:K16], fm8 fp8 [K16:CK])
            atts = {}   # b -> attT tile

            def load(b, split=False, eng=None):
                eng = eng or nc.sync
                fm = fmp.tile([P, CK, HW], FP16)
                lbt = lbp.tile([P, LK, HW], FP16)
                fm16 = fmb.tile([P, K16, HW], BF16, tag="fm16", name="fm16")
                fm8 = fmb.tile([P, K8, HW], FP8, tag="fm8", name="fm8")
                if split:
                    # batch 0: land the bf16 slice of fm first so the value
                    # GEMM starts while the rest of the startup DMA drains;
                    # the fp8 slice lands in 2 halves so DoubleRow matmuls
                    # chase the DMA instead of waiting for the whole tensor
                    eng.dma_start(fm[:, 0:K16, :], fm_d[b][:, 0:K16, :])
                    eng.dma_start(lbt[:], lb_d[b])
                    eng.dma_start(wlt[:], wlt_d.ap())
                    nc.scalar.copy(fm16[:], fm[:, 0:K16, :])
                    half = K8 // 2
                    for h in range(2):
                        sl = slice(K16 + h * half, K16 + (h + 1) * half)
                        eng.dma_start(fm[:, sl, :], fm_d[b][:, sl, :])
                        nc.scalar.copy(fm8[:, h * half:(h + 1) * half, :],
                                       fm[:, sl, :])
                else:
                    eng.dma_start(lbt[:], lb_d[b])
                    eng.dma_start(fm[:], fm_d[b])
                    # conversions on ScalarE: keeps the DVE free for the
                    # epilogue chain that paces the inter-batch handoff
                    nc.scalar.copy(fm16[:], fm[:, 0:K16, :])
                    nc.scalar.copy(fm8[:], fm[:, K16:CK, :])
                fms[b] = (fm, fm16, fm8)
                return lbt

            def att_lqt(b, lbt):
                lqt = proj.tile([P, NT, D], F32R, tag="lqt", name="lqt")
                for nt in range(NT):
                    ps = ps_small.tile([P, D], F32, tag="ps", name="ps")
                    for k in range(LK):
                        nc.tensor.matmul(
                            ps[:], lbt[:, k, nt * P:(nt + 1) * P], wlt[:, k, :],
                            start=(k == 0),
                            stop=(not with_bias and k == LK - 1))
                    if with_bias:
                        nc.tensor.matmul(ps[:], ones[:], blb[:], start=False,
                                         stop=True)
                    nc.vector.tensor_copy(lqt[:, nt, :], ps[:])
                return lqt

            def att_fkt(b, nt, fkt=None):
                if fkt is None:
                    fkt = proj.tile([P, NT, D], F32R, tag="fkt", name="fkt")
                fm = fms[b][0]
                ps = ps_small.tile([P, D], F32, tag="ps", name="ps")
                for k in range(CK):
                    nc.tensor.matmul(
                        ps[:], fm[:, k, nt * P:(nt + 1) * P], wft[:, k, :],
                        start=(k == 0),
                        stop=(not with_bias and k == CK - 1))
                if with_bias:
                    nc.tensor.matmul(ps[:], ones[:], bfb[:], start=False,
                                     stop=True)
                nc.vector.tensor_copy(fkt[:, nt, :], ps[:])
                return fkt

            def att_softmax(b, lqt, fkt):
                att = attp.tile([P, NT, D], BF16, tag="att", name="att")
                negmax = stat.tile([P, NT], F32, tag="negmax", name="negmax")
                sumexp = stat.tile([P, NT], F32, tag="sumexp", name="sumexp")
                recip = stat.tile([P, NT], F32, tag="recip", name="recip")
                for dm in range(NT):
                    ps = ps_small.tile([P, D], F32, tag="ps", name="ps")
                    for kn in range(NT):
                        nc.tensor.matmul(
                            ps[:], lqt[:, kn, dm * P:(dm + 1) * P], fkt[:, kn, :],
                            start=(kn == 0), stop=(kn == NT - 1))
                    nc.vector.tensor_reduce(
                        negmax[:, dm:dm + 1], ps[:], axis=mybir.AxisListType.X,
                        op=mybir.AluOpType.max, negate=True)
                    nc.scalar.activation(
                        att[:, dm, :], ps[:], mybir.ActivationFunctionType.Exp,
                        bias=negmax[:, dm:dm + 1], scale=1.0,
                        accum_out=sumexp[:, dm:dm + 1])
                    nc.vector.reciprocal(recip[:, dm:dm + 1], sumexp[:, dm:dm + 1])
                    nc.vector.tensor_scalar_mul(
                        att[:, dm, :], att[:, dm, :], recip[:, dm:dm + 1])

                attT = attp.tile([P, NT, D], BF16, tag="attT", name="attT")
                for et in range(NT):
                    psT = ps_att.tile([P, D], BF16, name="psT")
                    for dt_ in range(NT):
                        nc.tensor.transpose(
                            psT[:, dt_ * P:(dt_ + 1) * P],
                            att[:, dt_, et * P:(et + 1) * P], ident[:])
                    nc.scalar.copy(attT[:, et, :], psT[:])
                atts[b] = attT

            def att_path(b, lbt):
                """fkT/lqT/logits/softmax/transpose -> attT (PE: ~5us)."""
                lqt = att_lqt(b, lbt)
                fkt = att_fkt(b, 0)
                att_fkt(b, 1, fkt)
                att_softmax(b, lqt, fkt)

            def final_pair(b, fm, vt, attT, cm, dma_eng=None):
                """out chunks cm, cm+1: matmuls + fused epilogue + one DMA."""
                dma_eng = dma_eng or nc.gpsimd
                ost = outp.tile([P, 2, D], F32, name="ost")
                for j in range(2):
                    ps = ps_small.tile([P, D], F32, tag="ps", name="fps")
                    for kn in range(NT):
                        nc.tensor.matmul(
                            ps[:], vt[:, kn, (cm + j) * P:(cm + j + 1) * P],
                            attT[:, kn, :],
                            start=(kn == 0), stop=(kn == NT - 1))
                    nc.vector.scalar_tensor_tensor(
                        ost[:, j, :], ps[:], bvb[:, cm + j:cm + j + 1],
                        fm[:, cm + j, :],
                        op0=mybir.AluOpType.add, op1=mybir.AluOpType.add)
                dma_eng.dma_start(out_d[b][:, cm:cm + 2, :], ost[:])

            def vt_copy(vt, nt, cc, ps):
                """PSUM -> SBUF bf16 with the 1/VSCALE compensation; alternate
                ScalarE / DVE so consecutive bank drains run in parallel."""
                dst = vt[:, nt, cc * VC:(cc + 1) * VC]
                if cc % 2 == 0:
                    nc.scalar.mul(dst, ps[:], 1.0 / VSCALE)
                else:
                    nc.vector.tensor_scalar_mul(dst, ps[:], 1.0 / VSCALE)

            def value_gemm_nt(fm16, fm8, vt, nt):
                """vT[:, nt, :]: k-outer / cc-inner so each stationary
                activation tile is reused across the 4 cc chunks (amortizes
                LDWEIGHTS, critical for the fp8 DoubleRow section whose
                256-col weight loads are 2x the bf16 cost)."""
                pss = [ps_val.tile([P, VC], F32, tag="vps", name=f"vps{i}")
                       for i in range(C // VC)]
                for k in range(K16):
                    for cc in range(C // VC):
                        nc.tensor.matmul(
                            pss[cc][:], fm16[:, k, nt * P:(nt + 1) * P],
                            wvt16[:, k, cc * VC:(cc + 1) * VC],
                            start=(k == 0), stop=False)
                for kp in range(K8 // 2):
                    for cc in range(C // VC):
                        nc.tensor.matmul(
                            pss[cc][:],
                            fm8[:, 2 * kp:2 * kp + 2, nt * P:(nt + 1) * P],
                            wvt8[:, 2 * kp:2 * kp + 2, cc * VC:(cc + 1) * VC],
                            start=False, stop=(kp == K8 // 2 - 1),
                            perf_mode=DR)
                for cc in range(C // VC):
                    vt_copy(vt, nt, cc, pss[cc])

            def value_out(b, fm, vt, attT):
                """out = vT.T @ attT + bv + residual, interleaved with the
                next batch's attention path when available."""
                nxt = interleave.pop(b, None)
                if nxt is None:
                    for cm in range(0, CM, 2):
                        final_pair(b, fm, vt, attT, cm)
                else:
                    # interleave final pairs (DVE-chain-paced) with the next
                    # batch's attention matmuls (PE-paced) to fill PE stalls
                    nb, nlbt = nxt
                    final_pair(b, fm, vt, attT, 0)
                    final_pair(b, fm, vt, attT, 2)
                    lqt = att_lqt(nb, nlbt)
                    final_pair(b, fm, vt, attT, 4)
                    final_pair(b, fm, vt, attT, 6)
                    fkt = att_fkt(nb, 0)
                    final_pair(b, fm, vt, attT, 8)
                    final_pair(b, fm, vt, attT, 10)
                    att_fkt(nb, 1, fkt)
                    final_pair(b, fm, vt, attT, 12)
                    final_pair(b, fm, vt, attT, 14)
                    att_softmax(nb, lqt, fkt)

            def value_final(b):
                """vT (big GEMM) + out epilogue."""
                fm, fm16, fm8 = fms.pop(b)
                attT = atts.pop(b)
                vt = valp.tile([P, NT, C], BF16, name="vt")
                for nt in range(NT):
                    value_gemm_nt(fm16, fm8, vt, nt)
                value_out(b, fm, vt, attT)

            def value_final_tail(b):
                """Last batch: cc-outer so each vT column chunk finishes early
                and its out chunks interleave with the remaining value GEMM,
                shrinking the serial tail."""
                fm, fm16, fm8 = fms.pop(b)
                attT = atts.pop(b)
                vt = valp.tile([P, NT, C], BF16, name="vt")
                chunks = [(cc * VC, VC) for cc in range(C // VC)]
                engs = [nc.sync, nc.scalar, nc.gpsimd]
                ei = 0
                for off, width in chunks:
                    pss = [ps_val.tile([P, width], F32, tag="vps",
                                       name=f"vps{i}") for i in range(NT)]
                    for k in range(K16):
                        for nt in range(NT):
                            nc.tensor.matmul(
                                pss[nt][:], fm16[:, k, nt * P:(nt + 1) * P],
                                wvt16[:, k, off:off + width],
                                start=(k == 0), stop=False)
                    for kp in range(K8 // 2):
                        for nt in range(NT):
                            nc.tensor.matmul(
                                pss[nt][:],
                                fm8[:, 2 * kp:2 * kp + 2, nt * P:(nt + 1) * P],
                                wvt8[:, 2 * kp:2 * kp + 2, off:off + width],
                                start=False, stop=(kp == K8 // 2 - 1),
                                perf_mode=DR)
                    for nt in range(NT):
                        vt_copy(vt, nt, off // VC, pss[nt])
                    for cm in range(off // P, (off + width) // P, 2):
                        # spread descriptor generation over idle engines so
                        # the drain chain is not serialized on one sequencer
                        final_pair(b, fm, vt, attT, cm, dma_eng=engs[ei % 3])
                        ei += 1

            # software pipeline: attention path runs one batch ahead of the
            # big value GEMM so PE never waits on softmax. Batch 0's value
            # GEMM is emitted first and chases the weight DMA chunk-by-chunk;
            # batch 1's inputs ride the gpsimd ring BEHIND the weights so
            # they do not steal bandwidth from the startup critical path.
            lbt0 = load(0, split=True)
            nc.sync.dma_start(bvb[:], bv_d.ap())
            for k in range(K16):
                nc.gpsimd.dma_start(wvt16[:, k, :], wvt16_d.ap()[:, k, :])
            for kp in range(K8 // 2):
                nc.gpsimd.dma_start(wvt8[:, 2 * kp:2 * kp + 2, :],
                                    wvt8_d.ap()[:, 2 * kp:2 * kp + 2, :])
            nc.sync.dma_start(wft[:], wft_d.ap())
            interleave = {}
            lbt1 = load(1) if b_shard > 1 else None
            # warm the PE HAM clock gate during the initial weight-DMA window.
            # K=128 matmuls on the identity tile keep the full array active
            # (K=1 dummies do not register as PE-busy for the HAM).
            warm = ps_att.tile([P, P], F32, tag="psT", name="warm")
            for _ in range(warm_mms if b_shard > 1 else 0):
                nc.tensor.matmul(warm[:], ident[:], ident[:, :P], start=True,
                                 stop=True)
            # batch 0 unrolled: value GEMM before the attention path so the
            # PE consumes weight chunks the moment they land
            fm0, fm016, fm08 = fms[0]
            vt0 = valp.tile([P, NT, C], BF16, name="vt")
            for nt in range(NT):
                value_gemm_nt(fm016, fm08, vt0, nt)
            att_path(0, lbt0)
            if b_shard == 1:
                fm, _, _ = fms.pop(0)
                value_out(0, fm, vt0, atts.pop(0))
            else:
                interleave[0] = (1, lbt1)
                fm, _, _ = fms.pop(0)
                value_out(0, fm, vt0, atts.pop(0))
                for b in range(1, b_shard):
                    if b + 1 < b_shard:
                        lbt = load(b + 1)
                        interleave[b] = (b + 1, lbt)
                    if b == b_shard - 1:
                        value_final_tail(b)
                    else:
                        value_final(b)

    nc.compile()
    return nc


_NC_CACHE = {}


def _get_nc(b_shard, with_bias=True):
    key = (b_shard, with_bias)
    if key not in _NC_CACHE:
        _NC_CACHE[key] = build_kernel(b_shard, with_bias=with_bias)
    return _NC_CACHE[key]


def make_in_maps(feature_maps, labels, Wf, bf, Wl, bl, Wv, bv, b_shard=B_SHARD,
                 n_cores=N_CORES):
    def to_pkf(a, kt):
        # [rows=kt*P, free] -> [P, kt, free], partition-major for 1-line DMAs
        return np.ascontiguousarray(
            a.reshape(kt, P, a.shape[-1]).transpose(1, 0, 2))

    fm = np.asarray(feature_maps, dtype=np.float32).reshape(B, C, HW)
    fm = np.ascontiguousarray(
        fm.reshape(B, CK, P, HW).transpose(0, 2, 1, 3)).astype(np.float16)
    lb = np.asarray(labels, dtype=np.float32).reshape(B, L, HW)
    lb = np.ascontiguousarray(
        lb.reshape(B, LK, P, HW).transpose(0, 2, 1, 3)).astype(np.float16)
    wft = to_pkf(np.asarray(Wf, dtype=np.float32).T.astype(np.float16), CK)
    wlt = to_pkf(np.asarray(Wl, dtype=np.float32).T.astype(np.float16), LK)
    wvs = np.asarray(Wv, dtype=np.float32).T * VSCALE   # [c, o], pre-scaled
    wvt16 = to_pkf(wvs[:K16 * P].astype(ml_dtypes.bfloat16), K16)
    wvt8 = to_pkf(wvs[K16 * P:].astype(ml_dtypes.float8_e4m3), K8)
    bfr = np.asarray(bf, dtype=np.float32).reshape(1, D)
    blr = np.asarray(bl, dtype=np.float32).reshape(1, D)
    bvr = np.ascontiguousarray(
        np.asarray(bv, dtype=np.float32).reshape(CM, P).T)
    in_maps = []
    for i in range(n_cores):
        s = slice(i * b_shard, (i + 1) * b_shard)
        in_maps.append({
            "fm": fm[s], "lb": lb[s], "wft": wft, "wlt": wlt,
            "wvt16": wvt16, "wvt8": wvt8,
            "bfc": bfr, "blc": blr, "bvc": bvr,
            "ones": np.ones((1, P), dtype=np.float32),
        })
    return in_maps


def kernel(feature_maps, labels, Wf, bf, Wl, bl, Wv, bv, _trace=False,
           _tmpdir=None):
    with_bias = bool(np.any(np.asarray(bf)) or np.any(np.asarray(bl)))
    nc = _get_nc(B_SHARD, with_bias)
    in_maps = make_in_maps(feature_maps, labels, Wf, bf, Wl, bl, Wv, bv)
    res = run_bass_kernel_spmd(nc, in_maps, core_ids=list(range(N_CORES)),
                               trace=_trace, tmpdir=_tmpdir)
    out = np.concatenate([res.results[i]["out"] for i in range(N_CORES)], axis=0)
    kernel.last_exec_time_ns = res.exec_time_ns
    # [B, P, CK, HW] -> [B, C, H, W]
    out = out.transpose(0, 2, 1, 3).reshape(B, C, 16, 16)
    return np.ascontiguousarray(out).astype(np.float32)
